# revision 15
# baseline (speedup 1.0000x reference)
"""MoE gate (softmax + bias-adjusted top-8 routing) Trainium2 Bass kernel.

Full inputs in, full outputs out. Token dim (B*S = 32768) is sharded 8 ways
across NeuronCores; the tiny gate weight [E,H] and expert biases [E] are
replicated.

v5 design:
  - fp16 hi/lo matmul: x is host-split into xh = fp16(x), xl = fp16(x-xh)
    (exact to ~2^-23 together) and W^T chunks are packed [Wh_k | Wl_k]
    [128h, 128] fp16 stationary. Streaming xh then xl into one PSUM
    [128, 512] accumulates top = (xh+xl)@Wh, bottom = (xh+xl)@Wl, so
    top+bottom is the exact fp32-grade logit. fp16 streams at 1 col/cycle
    (~220ns per 512-token chunk MM) vs fp32's 2 passes (~858ns).
  - The half-sum + transpose happen in ONE data-stationary matmul per
    128-token tile: out[t,e] = sum_k psum_copy[k,t]*Sel[k,e] with
    Sel = [I64; I64] (host input), yielding token-major logits [128t,64e].
  - exp + per-subtile softmax denominator fused on ACT (accum_out).
  - Ranking by y = exp + S*b, order-equivalent to softmax+bias (S>0).
    The device emits idx = top-8 indices, v = max8(y) values, and S;
    the tiny host epilogue recovers the winners' softmax probs
    p = v/S - b[idx] and normalizes (exact; no device-side gather).
  - Software-pipelined emission: PE runs group g's logits while group
    g-1's selector matmuls wait on their ACT copy. x loads split across
    the SP hardware DGE ring (hi) and the gpsimd software ring (lo) so
    no single ring serializes the 4MB/group input stream; the ACT ring
    carries only the small result stores.
"""

import os
import sys
from contextlib import ExitStack

import numpy as np

sys.path.insert(0, "/opt/trn_rl_repo")

import concourse.bacc as bacc
import concourse.bass as bass
import concourse.mybir as mybir
import concourse.tile as tile

B, S, H, E, K = 8, 4096, 2048, 64, 8
N_CORES = 8
T = B * S
T_CORE = T // N_CORES  # 4096 tokens per core
TG = 512               # tokens per group
NSUB = TG // 128       # 128-token subtiles per group
KH = H // 128          # contraction chunks

f32 = mybir.dt.float32
f16 = mybir.dt.float16
i32 = mybir.dt.int32
u32 = mybir.dt.uint32
Alu = mybir.AluOpType
Act = mybir.ActivationFunctionType
Ax = mybir.AxisListType


def build_nc(t_core=T_CORE, repeat=1):
    G = t_core // TG
    nc = bacc.Bacc("TRN2", target_bir_lowering=False, debug=False,
                   enable_asserts=False)
    xi = nc.dram_tensor("xi", [H, 2 * t_core], f16, kind="ExternalInput").ap()
    whl = nc.dram_tensor("whl", [H, 2 * E], f16, kind="ExternalInput").ap()
    sel = nc.dram_tensor("sel", [128, E], f32, kind="ExternalInput").ap()
    eb = nc.dram_tensor("eb", [E], f32, kind="ExternalInput").ap()
    # Outputs in [128p, G, ...] layout (token = (g*NSUB+j)*128 + p) so the
    # store DMAs are fully contiguous; host reorders the tiny results.
    idx_out = nc.dram_tensor("idx_out", [128, G, NSUB, K], u32,
                             kind="ExternalOutput").ap()
    v_out = nc.dram_tensor("v_out", [128, G, NSUB, K], f32,
                           kind="ExternalOutput").ap()
    s_out = nc.dram_tensor("s_out", [128, G, NSUB], f32,
                           kind="ExternalOutput").ap()

    with tile.TileContext(nc) as tc:
        with ExitStack() as ctx:
            _emit(ctx, tc, nc, xi, whl, sel, eb,
                  idx_out, v_out, s_out, G, repeat)
    nc.compile()
    return nc


def _emit(ctx, tc, nc, xi, whl, sel, eb, idx_out, v_out, s_out, G,
          repeat=1):
    const = ctx.enter_context(tc.tile_pool(name="const", bufs=1))
    xtp = ctx.enter_context(tc.tile_pool(name="xtp", bufs=3))
    psl = ctx.enter_context(tc.tile_pool(name="psl", bufs=2, space="PSUM"))
    pst = ctx.enter_context(tc.tile_pool(name="pst", bufs=2, space="PSUM"))
    wk = ctx.enter_context(tc.tile_pool(name="wk", bufs=2))
    outp = ctx.enter_context(tc.tile_pool(name="outp", bufs=3))

    # Constants: packed [Wh|Wl] chunks (stationary), selector [I;I],
    # broadcast biases.
    whl_sb = const.tile([128, KH, 2 * E], f16)
    nc.sync.dma_start(out=whl_sb,
                      in_=whl.rearrange("(k p) e -> p k e", p=128))
    sel_sb = const.tile([128, E], f32)
    nc.sync.dma_start(out=sel_sb, in_=sel)
    bias_sb = const.tile([128, E], f32)
    nc.gpsimd.dma_start(out=bias_sb, in_=eb.unsqueeze(0).broadcast_to((128, E)))

    # PE matmuls lower to LDW+MM structs that can carry only ONE sync wait.
    # Consume the W/sel DMA deps with single-wait PE warmup ops so loop
    # matmuls each need at most one (their x-tile DMA).
    scr = ctx.enter_context(tc.tile_pool(name="scr", bufs=1, space="PSUM"))
    warm_m = scr.tile([64, 64], f32, tag="warm_m")
    nc.tensor.matmul(warm_m, lhsT=whl_sb[:, 0, 0:E], rhs=whl_sb[:, 0, 0:E],
                     start=True, stop=True)
    nc.tensor.matmul(warm_m, lhsT=sel_sb[:, 0:64], rhs=sel_sb, start=True,
                     stop=True)
    # pre-consume the bias broadcast on the engine that reads it (DVE)
    warm_v = const.tile([128, 1], f32, tag="warm_v")
    nc.vector.tensor_copy(warm_v, bias_sb[:, 0:1])

    # x ships as [hi 512-token block | lo 512-token block] per group so
    # every DMA partition line is 2KB (1KB f16 lines halve DMA efficiency)
    xi_r = xi.rearrange("(k p) (g u) -> g p k u", p=128, u=2 * TG)

    KHQ = KH // 2        # chunks per x sub-DMA

    state = {}

    def stage_a(g):
        # ---- load the group's [hi|lo] chunk tiles; the first chunk half
        # rides the SP hardware ring, the second the ACT hardware ring,
        # so the two 2MB halves transfer concurrently.
        xa = xtp.tile([128, KHQ, 2 * TG], f16, tag="xa")
        nc.sync.dma_start(out=xa, in_=xi_r[g][:, 0:KHQ, :])
        xb = xtp.tile([128, KHQ, 2 * TG], f16, tag="xb")
        nc.sync.dma_start(out=xb, in_=xi_r[g][:, KHQ:KH, :])

        def xs(k, lo):
            t = xa if k < KHQ else xb
            kk = k if k < KHQ else k - KHQ
            return t[:, kk, TG:2 * TG] if lo else t[:, kk, 0:TG]

        # ---- [x@Wh ; x@Wl] halves accumulate in one PSUM [128, 512]
        ps_c = psl.tile([128, TG], f32, tag="ps_c")
        for k in range(KH):
            nc.tensor.matmul(ps_c, lhsT=whl_sb[:, k, :], rhs=xs(k, False),
                             start=(k == 0), stop=False)
        for k in range(KH):
            nc.tensor.matmul(ps_c, lhsT=whl_sb[:, k, :], rhs=xs(k, True),
                             start=False, stop=(k == KH - 1))
        state[g] = ps_c

    def stage_b(g):
        ps_c = state.pop(g)
        # ---- copy halves out of PSUM; selector matmul fuses top+bottom sum
        # with the transpose to token-major logits [128t, 64e]
        pc = wk.tile([128, TG], f32, tag="pc")
        nc.scalar.copy(pc, ps_c)
        ps_t = pst.tile([128, NSUB, E], f32, tag="ps_t")
        for j in range(NSUB):
            nc.tensor.matmul(ps_t[:, j, :],
                             lhsT=pc[:, j * 128:(j + 1) * 128],
                             rhs=sel_sb, start=True, stop=True)
        # ---- exp + fused per-subtile softmax denominator S
        sr = wk.tile([128, NSUB, E], f32, tag="sr")
        S_ = outp.tile([128, NSUB], f32, tag="S")
        for j in range(NSUB):
            nc.scalar.activation(sr[:, j, :], ps_t[:, j, :], func=Act.Exp,
                                 accum_out=S_[:, j:j + 1])

        # ---- rank by y = exp + S*b (== S * (softmax + bias), same order)
        y_ = wk.tile([128, NSUB, E], f32, tag="y")
        v_ = outp.tile([128, NSUB, K], f32, tag="v")
        idx_g = outp.tile([128, NSUB, K], u32, tag="idx_g")
        for j in range(NSUB):
            nc.vector.scalar_tensor_tensor(y_[:, j, :], bias_sb,
                                           S_[:, j:j + 1], sr[:, j, :],
                                           Alu.mult, Alu.add)
            nc.vector.max(out=v_[:, j, :], in_=y_[:, j, :])
            nc.vector.max_index(out=idx_g[:, j, :], in_max=v_[:, j, :],
                                in_values=y_[:, j, :])

        # per-group stores overlap with later groups' compute; the ACT DGE
        # ring carries only these small results
        nc.scalar.dma_start(out=idx_out[:, g], in_=idx_g)
        nc.scalar.dma_start(out=v_out[:, g], in_=v_)
        nc.scalar.dma_start(out=s_out[:, g], in_=S_)

    # software-pipelined emission: stage_b(g-1) lands between the logits
    # matmul runs of g and g+1, so PE never waits on ACT mid-stream
    order = [g for _ in range(repeat) for g in range(G)]
    for i, g in enumerate(order):
        stage_a(g)
        if i > 0:
            stage_b(order[i - 1])
    stage_b(order[-1])


_NC_CACHE = {}


def get_nc(t_core=T_CORE, repeat=1):
    key = (t_core, repeat)
    if key not in _NC_CACHE:
        _NC_CACHE[key] = build_nc(t_core, repeat)
    return _NC_CACHE[key]


def _reorder(dev_out, t_core):
    # [128, G, NSUB, ...] -> [t_core, ...] with token = (g*NSUB+j)*128 + p
    d = dev_out
    rest = d.shape[3:]
    return d.transpose(1, 2, 0, 3).reshape((t_core,) + rest) if rest else \
        d.transpose(1, 2, 0).reshape(t_core)


def kernel(hidden_states, weight, expert_biases, top_k):
    from concourse.bass_utils import run_bass_kernel_spmd

    assert int(top_k) == K
    x2d = np.asarray(hidden_states, dtype=np.float32).reshape(-1, H)
    w32 = np.asarray(weight, dtype=np.float32).T          # [H, E]
    wh = w32.astype(np.float16)
    wl = (w32 - wh.astype(np.float32)).astype(np.float16)
    whl = np.ascontiguousarray(np.concatenate([wh, wl], axis=1))  # [H, 2E]
    selm = np.ascontiguousarray(
        np.vstack([np.eye(E, dtype=np.float32)] * 2))     # [128, E]
    eb = np.ascontiguousarray(np.asarray(expert_biases, dtype=np.float32))

    nc = get_nc()
    Gc = T_CORE // TG
    in_maps = []
    for c in range(N_CORES):
        xc = x2d[c * T_CORE:(c + 1) * T_CORE, :].T        # [H, T_CORE] view
        xch = np.asarray(xc, dtype=np.float16)
        xcl = (xc - xch.astype(np.float32)).astype(np.float16)
        xi = np.stack([xch.reshape(H, Gc, TG), xcl.reshape(H, Gc, TG)],
                      axis=2).reshape(H, 2 * T_CORE)
        in_maps.append({"xi": np.ascontiguousarray(xi), "whl": whl,
                        "sel": selm, "eb": eb})
    res = run_bass_kernel_spmd(nc, in_maps, core_ids=list(range(N_CORES)))

    idxs, vs, ss = [], [], []
    for c in range(N_CORES):
        r = res.results[c]
        idxs.append(_reorder(r["idx_out"], T_CORE))
        vs.append(_reorder(r["v_out"], T_CORE))
        ss.append(_reorder(r["s_out"], T_CORE).reshape(T_CORE))
    idx = np.concatenate(idxs, axis=0)                    # [T, K] u32
    v = np.concatenate(vs, axis=0).astype(np.float64)     # [T, K]
    s = np.concatenate(ss, axis=0).astype(np.float64)     # [T]
    # host epilogue: winners' softmax probs p = v/S - b[idx], normalized
    p = v / s[:, None] - eb.astype(np.float64)[idx]
    w = p / (p.sum(axis=1, keepdims=True) + 1e-20)
    return idx.astype(np.int32), w.astype(np.float32)


# revision 16
# speedup vs baseline: 1.0813x; 1.0813x over previous
"""MoE gate (softmax + bias-adjusted top-8 routing) Trainium2 Bass kernel.

Full inputs in, full outputs out. Token dim (B*S = 32768) is sharded 8 ways
across NeuronCores; the tiny gate weight [E,H] and expert biases [E] are
replicated.

v5 design:
  - fp16 hi/lo matmul: x is host-split into xh = fp16(x), xl = fp16(x-xh)
    (exact to ~2^-23 together) and W^T chunks are packed [Wh_k | Wl_k]
    [128h, 128] fp16 stationary. Streaming xh then xl into one PSUM
    [128, 512] accumulates top = (xh+xl)@Wh, bottom = (xh+xl)@Wl, so
    top+bottom is the exact fp32-grade logit. fp16 streams at 1 col/cycle
    (~220ns per 512-token chunk MM) vs fp32's 2 passes (~858ns).
  - The half-sum + transpose happen in ONE data-stationary matmul per
    128-token tile: out[t,e] = sum_k psum_copy[k,t]*Sel[k,e] with
    Sel = [I64; I64] (host input), yielding token-major logits [128t,64e].
  - exp + per-subtile softmax denominator fused on ACT (accum_out).
  - Ranking by y = exp + S*b, order-equivalent to softmax+bias (S>0).
    The device emits idx = top-8 indices, v = max8(y) values, and S;
    the tiny host epilogue recovers the winners' softmax probs
    p = v/S - b[idx] and normalizes (exact; no device-side gather).
  - Software-pipelined emission: PE runs group g's logits while group
    g-1's selector matmuls wait on their ACT copy. x loads split across
    the SP hardware DGE ring (hi) and the gpsimd software ring (lo) so
    no single ring serializes the 4MB/group input stream; the ACT ring
    carries only the small result stores.
"""

import os
import sys
from contextlib import ExitStack

import numpy as np

sys.path.insert(0, "/opt/trn_rl_repo")

import concourse.bacc as bacc
import concourse.bass as bass
import concourse.mybir as mybir
import concourse.tile as tile

B, S, H, E, K = 8, 4096, 2048, 64, 8
N_CORES = 8
T = B * S
T_CORE = T // N_CORES  # 4096 tokens per core
TG = 512               # tokens per group
NSUB = TG // 128       # 128-token subtiles per group
KH = H // 128          # contraction chunks

f32 = mybir.dt.float32
f16 = mybir.dt.float16
i32 = mybir.dt.int32
u32 = mybir.dt.uint32
Alu = mybir.AluOpType
Act = mybir.ActivationFunctionType
Ax = mybir.AxisListType


def build_nc(t_core=T_CORE, repeat=1):
    G = t_core // TG
    nc = bacc.Bacc("TRN2", target_bir_lowering=False, debug=False,
                   enable_asserts=False)
    xi = nc.dram_tensor("xi", [H, 2 * t_core], f16, kind="ExternalInput").ap()
    whl = nc.dram_tensor("whl", [H, 2 * E], f16, kind="ExternalInput").ap()
    sel = nc.dram_tensor("sel", [128, E], f32, kind="ExternalInput").ap()
    eb = nc.dram_tensor("eb", [E], f32, kind="ExternalInput").ap()
    # Outputs in [128p, G, ...] layout (token = (g*NSUB+j)*128 + p) so the
    # store DMAs are fully contiguous; host reorders the tiny results.
    idx_out = nc.dram_tensor("idx_out", [128, G, NSUB, K], u32,
                             kind="ExternalOutput").ap()
    v_out = nc.dram_tensor("v_out", [128, G, NSUB, K], f32,
                           kind="ExternalOutput").ap()
    s_out = nc.dram_tensor("s_out", [128, G, NSUB], f32,
                           kind="ExternalOutput").ap()

    with tile.TileContext(nc) as tc:
        with ExitStack() as ctx:
            _emit(ctx, tc, nc, xi, whl, sel, eb,
                  idx_out, v_out, s_out, G, repeat)
    nc.compile()
    return nc


def _emit(ctx, tc, nc, xi, whl, sel, eb, idx_out, v_out, s_out, G,
          repeat=1):
    const = ctx.enter_context(tc.tile_pool(name="const", bufs=1))
    xtp = ctx.enter_context(tc.tile_pool(name="xtp", bufs=4))
    psl = ctx.enter_context(tc.tile_pool(name="psl", bufs=3, space="PSUM"))
    pst = ctx.enter_context(tc.tile_pool(name="pst", bufs=2, space="PSUM"))
    wk = ctx.enter_context(tc.tile_pool(name="wk", bufs=2))
    outp = ctx.enter_context(tc.tile_pool(name="outp", bufs=3))

    # Constants: packed [Wh|Wl] chunks (stationary), selector [I;I],
    # broadcast biases.
    whl_sb = const.tile([128, KH, 2 * E], f16)
    nc.sync.dma_start(out=whl_sb,
                      in_=whl.rearrange("(k p) e -> p k e", p=128))
    sel_sb = const.tile([128, E], f32)
    nc.sync.dma_start(out=sel_sb, in_=sel)
    bias_sb = const.tile([128, E], f32)
    nc.gpsimd.dma_start(out=bias_sb, in_=eb.unsqueeze(0).broadcast_to((128, E)))

    # PE matmuls lower to LDW+MM structs that can carry only ONE sync wait.
    # Consume the W/sel DMA deps with single-wait PE warmup ops so loop
    # matmuls each need at most one (their x-tile DMA).
    scr = ctx.enter_context(tc.tile_pool(name="scr", bufs=1, space="PSUM"))
    warm_m = scr.tile([64, 64], f32, tag="warm_m")
    nc.tensor.matmul(warm_m, lhsT=whl_sb[:, 0, 0:E], rhs=whl_sb[:, 0, 0:E],
                     start=True, stop=True)
    nc.tensor.matmul(warm_m, lhsT=sel_sb[:, 0:64], rhs=sel_sb, start=True,
                     stop=True)
    # pre-consume the bias broadcast on the engine that reads it (DVE)
    warm_v = const.tile([128, 1], f32, tag="warm_v")
    nc.vector.tensor_copy(warm_v, bias_sb[:, 0:1])

    # x ships as [hi 512-token block | lo 512-token block] per group so
    # every DMA partition line is 2KB (1KB f16 lines halve DMA efficiency)
    xi_r = xi.rearrange("(k p) (g u) -> g p k u", p=128, u=2 * TG)

    KHQ = KH // 2        # chunks per x sub-DMA

    state = {}

    def stage_a(g):
        # ---- load the group's [hi|lo] chunk tiles; the first chunk half
        # rides the SP hardware ring, the second the ACT hardware ring,
        # so the two 2MB halves transfer concurrently.
        xa = xtp.tile([128, KHQ, 2 * TG], f16, tag="xa")
        nc.sync.dma_start(out=xa, in_=xi_r[g][:, 0:KHQ, :])
        xb = xtp.tile([128, KHQ, 2 * TG], f16, tag="xb")
        nc.sync.dma_start(out=xb, in_=xi_r[g][:, KHQ:KH, :])

        def xs(k, lo):
            t = xa if k < KHQ else xb
            kk = k if k < KHQ else k - KHQ
            return t[:, kk, TG:2 * TG] if lo else t[:, kk, 0:TG]

        # ---- [x@Wh ; x@Wl] halves accumulate in one PSUM [128, 512]
        ps_c = psl.tile([128, TG], f32, tag="ps_c")
        for k in range(KH):
            nc.tensor.matmul(ps_c, lhsT=whl_sb[:, k, :], rhs=xs(k, False),
                             start=(k == 0), stop=False)
        for k in range(KH):
            nc.tensor.matmul(ps_c, lhsT=whl_sb[:, k, :], rhs=xs(k, True),
                             start=False, stop=(k == KH - 1))
        state[g] = ps_c

    def stage_b(g):
        ps_c = state.pop(g)
        # ---- copy halves out of PSUM; selector matmul fuses top+bottom sum
        # with the transpose to token-major logits [128t, 64e]
        pc = wk.tile([128, TG], f32, tag="pc")
        nc.scalar.copy(pc, ps_c)
        ps_t = pst.tile([128, NSUB, E], f32, tag="ps_t")
        for j in range(NSUB):
            nc.tensor.matmul(ps_t[:, j, :],
                             lhsT=pc[:, j * 128:(j + 1) * 128],
                             rhs=sel_sb, start=True, stop=True)
        # ---- exp + fused per-subtile softmax denominator S
        sr = wk.tile([128, NSUB, E], f32, tag="sr")
        S_ = outp.tile([128, NSUB], f32, tag="S")
        for j in range(NSUB):
            nc.scalar.activation(sr[:, j, :], ps_t[:, j, :], func=Act.Exp,
                                 accum_out=S_[:, j:j + 1])

        # ---- rank by y = exp + S*b (== S * (softmax + bias), same order)
        y_ = wk.tile([128, NSUB, E], f32, tag="y")
        v_ = outp.tile([128, NSUB, K], f32, tag="v")
        idx_g = outp.tile([128, NSUB, K], u32, tag="idx_g")
        for j in range(NSUB):
            nc.vector.scalar_tensor_tensor(y_[:, j, :], bias_sb,
                                           S_[:, j:j + 1], sr[:, j, :],
                                           Alu.mult, Alu.add)
            nc.vector.max(out=v_[:, j, :], in_=y_[:, j, :])
            nc.vector.max_index(out=idx_g[:, j, :], in_max=v_[:, j, :],
                                in_values=y_[:, j, :])

        # per-group stores overlap with later groups' compute; the ACT DGE
        # ring carries only these small results
        nc.scalar.dma_start(out=idx_out[:, g], in_=idx_g)
        nc.scalar.dma_start(out=v_out[:, g], in_=v_)
        nc.scalar.dma_start(out=s_out[:, g], in_=S_)

    # software-pipelined emission: stage_b(g-1) lands between the logits
    # matmul runs of g and g+1, so PE never waits on ACT mid-stream
    order = [g for _ in range(repeat) for g in range(G)]
    for i, g in enumerate(order):
        stage_a(g)
        if i > 0:
            stage_b(order[i - 1])
    stage_b(order[-1])


_NC_CACHE = {}


def get_nc(t_core=T_CORE, repeat=1):
    key = (t_core, repeat)
    if key not in _NC_CACHE:
        _NC_CACHE[key] = build_nc(t_core, repeat)
    return _NC_CACHE[key]


def _reorder(dev_out, t_core):
    # [128, G, NSUB, ...] -> [t_core, ...] with token = (g*NSUB+j)*128 + p
    d = dev_out
    rest = d.shape[3:]
    return d.transpose(1, 2, 0, 3).reshape((t_core,) + rest) if rest else \
        d.transpose(1, 2, 0).reshape(t_core)


def kernel(hidden_states, weight, expert_biases, top_k):
    from concourse.bass_utils import run_bass_kernel_spmd

    assert int(top_k) == K
    x2d = np.asarray(hidden_states, dtype=np.float32).reshape(-1, H)
    w32 = np.asarray(weight, dtype=np.float32).T          # [H, E]
    wh = w32.astype(np.float16)
    wl = (w32 - wh.astype(np.float32)).astype(np.float16)
    whl = np.ascontiguousarray(np.concatenate([wh, wl], axis=1))  # [H, 2E]
    selm = np.ascontiguousarray(
        np.vstack([np.eye(E, dtype=np.float32)] * 2))     # [128, E]
    eb = np.ascontiguousarray(np.asarray(expert_biases, dtype=np.float32))

    nc = get_nc()
    Gc = T_CORE // TG
    in_maps = []
    for c in range(N_CORES):
        xc = x2d[c * T_CORE:(c + 1) * T_CORE, :].T        # [H, T_CORE] view
        xch = np.asarray(xc, dtype=np.float16)
        xcl = (xc - xch.astype(np.float32)).astype(np.float16)
        xi = np.stack([xch.reshape(H, Gc, TG), xcl.reshape(H, Gc, TG)],
                      axis=2).reshape(H, 2 * T_CORE)
        in_maps.append({"xi": np.ascontiguousarray(xi), "whl": whl,
                        "sel": selm, "eb": eb})
    res = run_bass_kernel_spmd(nc, in_maps, core_ids=list(range(N_CORES)))

    idxs, vs, ss = [], [], []
    for c in range(N_CORES):
        r = res.results[c]
        idxs.append(_reorder(r["idx_out"], T_CORE))
        vs.append(_reorder(r["v_out"], T_CORE))
        ss.append(_reorder(r["s_out"], T_CORE).reshape(T_CORE))
    idx = np.concatenate(idxs, axis=0)                    # [T, K] u32
    v = np.concatenate(vs, axis=0).astype(np.float64)     # [T, K]
    s = np.concatenate(ss, axis=0).astype(np.float64)     # [T]
    # host epilogue: winners' softmax probs p = v/S - b[idx], normalized
    p = v / s[:, None] - eb.astype(np.float64)[idx]
    w = p / (p.sum(axis=1, keepdims=True) + 1e-20)
    return idx.astype(np.int32), w.astype(np.float32)
